# revision 20
# baseline (speedup 1.0000x reference)
"""Trainium2 Bass kernel for nn_DeformableSliceGrouped.

Sharding: 8 cores = 2 batches x 4 h-quarters. Each core handles
(1, 256, 32, 12, 48) of the input in a (c, hw, z) "z-innermost" layout.

v2: PE-saturation restructure.
  Host: W2 = o_w@v_w, qpe = q_w@pe.T, w2pe = pe@W2.T precomputed on host.
  P1a: q GEMM per tile -> running per-z max on DVE (feeds B1 early)
  B1:  AllReduce-max qp within 4-core batch group (issued immediately)
  P1b: ALL v' GEMMs (x stationary -> v'T in SBUF fp16); the pe bias enters
       via one K=32 matmul per pair (zselw x w2pe2) accumulated in PSUM.
       Runs concurrently with the B1 collective -> PE never stalls on it.
  M:   offsets/attn/softmax -> 32x32 mixing matrix M -> block-diag w128
  P1c: ALL mix matmuls (z-on-partitions block-diag M.T); ACT copies
       PSUM->vbuf fp16; DVE bn_stats on the fp16 copy
  B2:  AllReduce-add BN stats over 8 cores; fold scale/shift
  P2b: normalize (ACT/DVE split) + residual add + fp16 DMA out
"""
from contextlib import ExitStack

import numpy as np

import concourse.bass as bass
import concourse.bacc as bacc
import concourse.tile as tile
import concourse.mybir as mybir
from concourse import bass_utils

F32 = mybir.dt.float32
F16 = mybir.dt.float16

B, C, Z, H, W = 2, 256, 32, 48, 48
HL = H // 4            # h rows per core
FL = HL * W            # 576 hw positions per core
FPT = 16               # hw positions per tile
G = FL // FPT          # 36 tiles
NHP = 6                # heads*points
NLOC = FL * Z          # per-core elements per channel (18432)
NTOT = 8 * NLOC        # global elements per channel (147456)
EPS = 1e-5

QP_GROUPS = [[0, 1, 2, 3], [4, 5, 6, 7]]
ALL_GROUP = [[0, 1, 2, 3, 4, 5, 6, 7]]


def ts(i, n):
    return slice(i * n, (i + 1) * n)


def _emit(tc, t, dbg=False):
    nc = tc.nc
    ctx = ExitStack()
    cp = ctx.enter_context(tc.tile_pool(name="consts", bufs=1))
    bigp = ctx.enter_context(tc.tile_pool(name="big", bufs=1))
    smp = ctx.enter_context(tc.tile_pool(name="smalls", bufs=3))
    resp = ctx.enter_context(tc.tile_pool(name="res", bufs=3))
    dramp = ctx.enter_context(tc.tile_pool(name="dram", bufs=1, space="DRAM"))

    # ---- constant loads ----
    qwT = cp.tile([128, 2, C], F16)
    nc.sync.dma_start(qwT[:], t["qwT16"].rearrange("(h p) m -> p h m", p=128))
    w2T = cp.tile([128, 2, C], F16)
    nc.sync.dma_start(w2T[:], t["w2T16"].rearrange("(h p) m -> p h m", p=128))
    qpe = cp.tile([128, 2, Z], F32)
    nc.sync.dma_start(qpe[:], t["qpe"].rearrange("(h p) z -> p h z", p=128))
    sawT = cp.tile([128, 2, 12], F16)
    nc.sync.dma_start(sawT[:], t["sawT16"].rearrange("(h p) j -> p h j", p=128))
    ident = cp.tile([Z, Z], F16)
    nc.sync.dma_start(ident[:], t["ident32"][:])
    # zero-padded to K=128 so the PE tile config matches the x-chains
    zselw = cp.tile([128, 128], F16)
    nc.sync.dma_start(zselw[:], t["zselw"][:])
    w2pe2 = cp.tile([128, 2 * C], F16)
    nc.sync.dma_start(w2pe2[:], t["w2pe2"][:])
    iota = cp.tile([Z, Z], F32)
    nc.sync.dma_start(iota[:], t["iota"][:])
    sab = cp.tile([Z, 12], F32)
    sab_src = bass.AP(tensor=t["sab"].tensor, offset=0, ap=[[0, Z], [1, 12]])
    nc.sync.dma_start(sab[:], sab_src)
    gam = cp.tile([128, 2, 1], F32)
    nc.sync.dma_start(gam[:], t["gb"].rearrange("(h p) j -> p h j", p=128)[:, :, 0:1])
    bet = cp.tile([128, 2, 1], F32)
    nc.sync.dma_start(bet[:], t["gb"].rearrange("(h p) j -> p h j", p=128)[:, :, 1:2])

    # big persistent regions
    x_all = bigp.tile([128, 2, G, FPT * Z], F16)
    vbuf = bigp.tile([128, 2, G, FPT * Z], F16)
    for gp in range(G // 2):
        for kh in range(2):
            nc.sync.dma_start(
                x_all[:, kh, ts(gp, 2), :],
                t["x16"][ts(kh, 128), ts(gp, 2 * FPT), :],
            )

    qmax = cp.tile([128, 2, Z], F32)
    nc.vector.memset(qmax[:], -3.0e38)
    w128 = cp.tile([128, 128], F16)
    nc.vector.memset(w128[:], 0.0)
    stats = cp.tile([128, 2, G, 6], F32)

    # ---- P1a: q GEMMs -> per-z max (feeds the qp collective early) ----
    with tc.tile_pool(name="p1q", bufs=2, space="PSUM") as p1q:
        for g in range(G):
            qps = p1q.tile([128, 2, FPT * Z], F32, tag="qps")
            # kh outer: consecutive matmuls hit alternating PSUM banks so the
            # PE can pipeline the next stationary load under the current MM
            for kh in range(2):
                for mh in range(2):
                    nc.tensor.matmul(
                        qps[:, mh, :], qwT[:, kh, ts(mh, 128)], x_all[:, kh, g, :],
                        start=(kh == 0), stop=(kh == 1),
                        skip_group_check=True,
                    )
            qred = smp.tile([128, 2, Z], F32, tag="qred")
            nc.vector.tensor_reduce(
                out=qred[:],
                in_=qps[:].rearrange("p h (f z) -> p h z f", z=Z),
                axis=mybir.AxisListType.X,
                op=mybir.AluOpType.max,
            )
            nc.vector.tensor_max(qmax[:], qmax[:], qred[:])

    # ---- B1 issue: qp allreduce (hidden under the v' GEMMs below) ----
    qp_loc = cp.tile([128, 2, Z], F32)
    nc.vector.tensor_add(qp_loc[:], qmax[:], qpe[:])
    qpin = dramp.tile([128, 2, Z], F32)
    qpout = dramp.tile([128, 2, Z], F32)
    nc.sync.dma_start(qpin[:], qp_loc[:])
    nc.gpsimd.collective_compute(
        "AllReduce", mybir.AluOpType.max, replica_groups=QP_GROUPS,
        ins=[qpin[:].opt()], outs=[qpout[:].opt()],
    )
    qp16 = cp.tile([128, 2, Z], F16)
    nc.gpsimd.dma_start(qp16[:], qpout[:])

    # ---- P1b: ALL v' GEMMs (independent of B1) ----
    pv = ctx.enter_context(tc.tile_pool(name="pv", bufs=2, space="PSUM"))
    for g in range(G):
        # both pair-chains live at once; instructions interleave across the
        # two PSUM banks so stationary loads pipeline under the previous MM
        vps = [
            pv.tile([128, 2, C], F32, tag=f"vps{pair}", name=f"vps{pair}")
            for pair in range(2)
        ]
        for pair in range(2):
            # W2@pe bias first (z-pattern repeats every 32 partitions):
            # start=True zeroes the full region, x-chains then accumulate.
            nc.tensor.matmul(
                vps[pair][:], zselw[:], w2pe2[:],
                start=True, stop=False, skip_group_check=True,
            )
        for i in range(2):
            for kh in range(2):
                for pair in range(2):
                    blk = pair * 2 + i
                    nc.tensor.matmul(
                        vps[pair][:, i, :], x_all[:, kh, g, ts(blk, 128)],
                        w2T[:, kh, :], start=False,
                        stop=(i == 1 and kh == 1),
                        skip_group_check=True,
                    )
        for pair in range(2):
            out_view = vbuf[:, :, g, ts(pair, 2 * 128)].rearrange(
                "p h (b c) -> p b h c", b=2
            )
            in_view = vps[pair][:].rearrange("p b (h c) -> p b h c", h=2)
            if pair == 0:
                nc.vector.tensor_copy(out_view, in_view)
            else:
                nc.scalar.copy(out_view, in_view)
        if dbg and g == 0:
            nc.sync.dma_start(t["dbg_v0"][:], vbuf[:, :, 0, :])

    # ---- M build (after B1): offsets/attn -> M -> block-diag w128 ----
    psml_ctx = tc.tile_pool(name="psml", bufs=1, space="PSUM")
    psml = psml_ctx.__enter__()
    sa_ps = psml.tile([Z, 12], F32, tag="small")
    for kh in range(2):
        nc.tensor.matmul(
            sa_ps[:], qp16[:, kh, :], sawT[:, kh, :],
            start=(kh == 0), stop=(kh == 1),
        )
    logits = cp.tile([Z, 12], F32)
    nc.vector.tensor_add(logits[:], sa_ps[:], sab[:])
    off = cp.tile([Z, NHP], F32)
    nc.vector.tensor_scalar(
        out=off[:], in0=logits[:, 0:NHP], scalar1=0.0, scalar2=float(Z - 1),
        op0=mybir.AluOpType.max, op1=mybir.AluOpType.min,
    )
    ex = cp.tile([Z, NHP], F32)
    nc.scalar.activation(ex[:], logits[:, NHP:12], mybir.ActivationFunctionType.Exp)
    ssum = cp.tile([Z, 1], F32)
    nc.vector.tensor_reduce(
        out=ssum[:], in_=ex[:], axis=mybir.AxisListType.X, op=mybir.AluOpType.add
    )
    rinv = cp.tile([Z, 1], F32)
    nc.vector.reciprocal(rinv[:], ssum[:])
    attn = cp.tile([Z, NHP], F32)
    nc.vector.tensor_scalar_mul(attn[:], ex[:], rinv[:, 0:1])

    # M[z, y] = sum_p attn[z,p] * relu(1 - |off[z,p] - y|)  (linear interp hat)
    msb = cp.tile([Z, Z], F32)
    mtmp = cp.tile([Z, Z], F32)
    dmy = cp.tile([Z, Z], F32)
    um = cp.tile([Z, Z], F32)
    vm = cp.tile([Z, Z], F32)
    for p in range(NHP):
        nc.vector.tensor_scalar(
            out=dmy[:], in0=iota[:], scalar1=off[:, p:p + 1], scalar2=None,
            op0=mybir.AluOpType.subtract,
        )
        nc.vector.tensor_scalar(
            out=um[:], in0=dmy[:], scalar1=-1.0, scalar2=1.0,
            op0=mybir.AluOpType.mult, op1=mybir.AluOpType.add,
        )
        nc.vector.tensor_scalar_add(vm[:], dmy[:], 1.0)
        nc.vector.tensor_tensor(
            out=um[:], in0=um[:], in1=vm[:], op=mybir.AluOpType.min
        )
        dst = msb if p == 0 else mtmp
        nc.vector.tensor_scalar(
            out=dst[:], in0=um[:], scalar1=0.0, scalar2=attn[:, p:p + 1],
            op0=mybir.AluOpType.max, op1=mybir.AluOpType.mult,
        )
        if p > 0:
            nc.vector.tensor_add(msb[:], msb[:], mtmp[:])
    m16 = cp.tile([Z, Z], F16)
    nc.vector.tensor_copy(m16[:], msb[:])
    mt_ps = psml.tile([Z, Z], F16, tag="small")
    nc.tensor.transpose(mt_ps[:], m16[:], ident[:])
    mt16 = cp.tile([Z, Z], F16)
    nc.vector.tensor_copy(mt16[:], mt_ps[:])
    for b4 in range(4):
        nc.vector.tensor_copy(w128[ts(b4, 32), ts(b4, 32)], mt16[:])
    psml_ctx.__exit__(None, None, None)

    # ---- P1c: ALL mix matmuls; ACT copy; DVE stats on fp16 copy ----
    pmix = ctx.enter_context(tc.tile_pool(name="pmix", bufs=2, space="PSUM"))
    for g in range(G):
        mix = pmix.tile([128, 2, FPT * Z], F32, tag="mix")
        for h in range(2):
            for blk in range(4):
                nc.tensor.matmul(
                    mix[:, h, ts(blk, 128)], vbuf[:, h, g, ts(blk, 128)],
                    w128[:], start=True, stop=True,
                )
        nc.scalar.copy(vbuf[:, :, g, :], mix[:])
        for h in range(2):
            nc.vector.bn_stats(stats[:, h, g, :], vbuf[:, h, g, :])
        if dbg and g == 0:
            nc.sync.dma_start(t["dbg_mix0"][:], vbuf[:, :, 0, :])

    # ---- B2: global BN stats ----
    mv = cp.tile([128, 2, 2], F32)
    for h in range(2):
        nc.vector.bn_aggr(mv[:, h, :], stats[:, h, :, :])
    msq = cp.tile([128, 2, 1], F32)
    nc.vector.tensor_mul(msq[:], mv[:, :, 0:1], mv[:, :, 0:1])
    ex2 = cp.tile([128, 2, 1], F32)
    nc.vector.tensor_add(ex2[:], mv[:, :, 1:2], msq[:])
    red_in = cp.tile([128, 2, 2], F32)
    nc.vector.tensor_scalar_mul(red_in[:, :, 0:1], mv[:, :, 0:1], float(NLOC))
    nc.vector.tensor_scalar_mul(red_in[:, :, 1:2], ex2[:], float(NLOC))
    rin = dramp.tile([128, 2, 2], F32)
    rout = dramp.tile([128, 2, 2], F32)
    nc.sync.dma_start(rin[:], red_in[:])
    nc.gpsimd.collective_compute(
        "AllReduce", mybir.AluOpType.add, replica_groups=ALL_GROUP,
        ins=[rin[:].opt()], outs=[rout[:].opt()],
    )
    gst = cp.tile([128, 2, 2], F32)
    nc.sync.dma_start(gst[:], rout[:])
    mean_g = cp.tile([128, 2, 1], F32)
    nc.vector.tensor_scalar_mul(mean_g[:], gst[:, :, 0:1], 1.0 / NTOT)
    ex2g = cp.tile([128, 2, 1], F32)
    nc.vector.tensor_scalar_mul(ex2g[:], gst[:, :, 1:2], 1.0 / NTOT)
    m2g = cp.tile([128, 2, 1], F32)
    nc.vector.tensor_mul(m2g[:], mean_g[:], mean_g[:])
    var_g = cp.tile([128, 2, 1], F32)
    nc.vector.tensor_sub(var_g[:], ex2g[:], m2g[:])
    eps_sb = cp.tile([128, 1], F32)
    nc.vector.memset(eps_sb[:], EPS)
    sd = cp.tile([128, 2, 1], F32)
    nc.scalar.activation(
        sd[:], var_g[:], mybir.ActivationFunctionType.Sqrt, bias=eps_sb[:]
    )
    rs = cp.tile([128, 2, 1], F32)
    nc.vector.reciprocal(rs[:], sd[:])
    s_sb = cp.tile([128, 2, 1], F32)
    nc.vector.tensor_mul(s_sb[:], rs[:], gam[:])
    ms_t = cp.tile([128, 2, 1], F32)
    nc.vector.tensor_mul(ms_t[:], mean_g[:], s_sb[:])
    t_sb = cp.tile([128, 2, 1], F32)
    nc.vector.tensor_sub(t_sb[:], bet[:], ms_t[:])
    if dbg:
        nc.sync.dma_start(t["dbg_qp"][:], qp16[:])
        nc.sync.dma_start(t["dbg_m"][:], msb[:])
        nc.sync.dma_start(t["dbg_w128"][:], w128[:])
        nc.sync.dma_start(t["dbg_s"][:], s_sb[:])
        nc.sync.dma_start(t["dbg_t"][:], t_sb[:])
        nc.sync.dma_start(t["dbg_mv"][:], mv[:])

    # ---- P2b: normalize (ACT/DVE) + residual (DVE) + fp16 store ----
    out_v = t["out"].rearrange("(h p) f z -> p h f z", p=128)
    for g in range(G):
        nrm = resp.tile([128, 2, FPT * Z], F16, tag="nrm")
        nc.scalar.activation(
            nrm[:, 0, :], vbuf[:, 0, g, :],
            mybir.ActivationFunctionType.Identity,
            scale=s_sb[:, 0, :], bias=t_sb[:, 0, :],
        )
        nc.vector.tensor_scalar(
            out=nrm[:, 1, :], in0=vbuf[:, 1, g, :],
            scalar1=s_sb[:, 1, :], scalar2=t_sb[:, 1, :],
            op0=mybir.AluOpType.mult, op1=mybir.AluOpType.add,
        )
        res = resp.tile([128, 2, FPT * Z], F16, tag="res")
        nc.gpsimd.tensor_add(res[:, 0, :], nrm[:, 0, :], x_all[:, 0, g, :])
        nc.vector.tensor_add(res[:, 1, :], nrm[:, 1, :], x_all[:, 1, g, :])
        for h in range(2):
            nc.sync.dma_start(
                out_v[:, h, ts(g, FPT), :],
                res[:, h, :].rearrange("p (f z) -> p f z", z=Z),
            )
    ctx.close()


_BUILT = None


def _build(dbg=False):
    global _BUILT
    if _BUILT is not None and not dbg:
        return _BUILT
    nc = bacc.Bacc("TRN2", target_bir_lowering=False, debug=False, num_devices=8)
    t = {}
    t["x16"] = nc.dram_tensor("x16", [C, FL, Z], F16, kind="ExternalInput").ap()
    t["qwT16"] = nc.dram_tensor("qwT16", [C, C], F16, kind="ExternalInput").ap()
    t["w2T16"] = nc.dram_tensor("w2T16", [C, C], F16, kind="ExternalInput").ap()
    t["qpe"] = nc.dram_tensor("qpe", [C, Z], F32, kind="ExternalInput").ap()
    t["sawT16"] = nc.dram_tensor("sawT16", [C, 12], F16, kind="ExternalInput").ap()
    t["sab"] = nc.dram_tensor("sab", [12], F32, kind="ExternalInput").ap()
    t["iota"] = nc.dram_tensor("iota", [Z, Z], F32, kind="ExternalInput").ap()
    t["ident32"] = nc.dram_tensor("ident32", [Z, Z], F16, kind="ExternalInput").ap()
    t["zselw"] = nc.dram_tensor("zselw", [128, 128], F16, kind="ExternalInput").ap()
    t["w2pe2"] = nc.dram_tensor("w2pe2", [128, 2 * C], F16, kind="ExternalInput").ap()
    t["gb"] = nc.dram_tensor("gb", [C, 2], F32, kind="ExternalInput").ap()
    t["out"] = nc.dram_tensor("out", [C, FL, Z], F16, kind="ExternalOutput").ap()
    if dbg:
        t["dbg_v0"] = nc.dram_tensor("dbg_v0", [128, 2, FPT * Z], F16, kind="ExternalOutput").ap()
        t["dbg_mix0"] = nc.dram_tensor("dbg_mix0", [128, 2, FPT * Z], F16, kind="ExternalOutput").ap()
        t["dbg_qp"] = nc.dram_tensor("dbg_qp", [128, 2, Z], F16, kind="ExternalOutput").ap()
        t["dbg_m"] = nc.dram_tensor("dbg_m", [Z, Z], F32, kind="ExternalOutput").ap()
        t["dbg_w128"] = nc.dram_tensor("dbg_w128", [128, 128], F16, kind="ExternalOutput").ap()
        t["dbg_s"] = nc.dram_tensor("dbg_s", [128, 2, 1], F32, kind="ExternalOutput").ap()
        t["dbg_t"] = nc.dram_tensor("dbg_t", [128, 2, 1], F32, kind="ExternalOutput").ap()
        t["dbg_mv"] = nc.dram_tensor("dbg_mv", [128, 2, 2], F32, kind="ExternalOutput").ap()
    with tile.TileContext(nc) as tc:
        _emit(tc, t, dbg=dbg)
    nc.compile()
    if not dbg:
        _BUILT = nc
    return nc


def _make_pe():
    pos = np.arange(Z, dtype=np.float32)[:, None]
    div = np.exp(np.arange(0, C, 2, dtype=np.float32) * (-np.log(10000.0) / C))
    pe = np.zeros((Z, C), dtype=np.float32)
    pe[:, 0::2] = np.sin(pos * div)
    pe[:, 1::2] = np.cos(pos * div)
    return pe


def _prepare_in_maps(features, q_w, v_w, o_w, offs_w, offs_b, attn_w, attn_b,
                     gamma, beta):
    features = np.ascontiguousarray(np.asarray(features, dtype=np.float32))
    pe = _make_pe()  # (Z, C) f32
    q_w = np.asarray(q_w, dtype=np.float32)
    W2 = np.asarray(o_w, dtype=np.float32) @ np.asarray(v_w, dtype=np.float32)
    w2pe = pe @ W2.T  # (Z, C)
    shared = {
        "qwT16": np.ascontiguousarray(q_w.T).astype(np.float16),
        "w2T16": np.ascontiguousarray(W2.T).astype(np.float16),
        "qpe": np.ascontiguousarray(q_w @ pe.T),
        "sawT16": np.concatenate(
            [np.asarray(offs_w).T, np.asarray(attn_w).T], axis=1
        ).astype(np.float16),
        "sab": np.concatenate(
            [np.asarray(offs_b), np.asarray(attn_b)]
        ).astype(np.float32),
        "iota": np.tile(np.arange(Z, dtype=np.float32)[None, :], (Z, 1)),
        "ident32": np.eye(Z, dtype=np.float16),
        "zselw": np.vstack(
            [np.tile(np.eye(Z, dtype=np.float16), (1, 128 // Z)),
             np.zeros((128 - Z, 128), np.float16)]
        ),
        "w2pe2": np.vstack(
            [np.tile(w2pe, (1, 2)).astype(np.float16),
             np.zeros((128 - Z, 2 * C), np.float16)]
        ),
        "gb": np.stack(
            [np.asarray(gamma, np.float32), np.asarray(beta, np.float32)], axis=1
        ),
    }
    in_maps = []
    for k in range(8):
        bi, hq = k // 4, k % 4
        xs = features[bi][:, :, hq * HL:(hq + 1) * HL, :]
        xs = np.ascontiguousarray(xs.transpose(0, 2, 3, 1)).reshape(C, FL, Z)
        m = dict(shared)
        m["x16"] = xs.astype(np.float16)
        in_maps.append(m)
    return in_maps


def kernel(**inputs):
    nc = _build()
    in_maps = _prepare_in_maps(**inputs)
    res = bass_utils.run_bass_kernel_spmd(nc, in_maps, core_ids=list(range(8)))

    full = np.empty((B, C, Z, H, W), dtype=np.float32)
    for k in range(8):
        bi, hq = k // 4, k % 4
        o = res.results[k]["out"].astype(np.float32).reshape(C, HL, W, Z).transpose(0, 3, 1, 2)
        full[bi][:, :, hq * HL:(hq + 1) * HL, :] = o
    return full


# revision 23
# speedup vs baseline: 1.3982x; 1.3982x over previous
"""Trainium2 Bass kernel for nn_DeformableSliceGrouped.

Sharding: 8 cores = 2 batches x 4 h-quarters. Each core handles
(1, 256, 32, 12, 48) of the input in a (c, hw, z) "z-innermost" layout.

v2: PE-saturation restructure.
  Host: W2 = o_w@v_w, qpe = q_w@pe.T, w2pe = pe@W2.T precomputed on host.
  P1a: q GEMM per tile -> running per-z max on DVE (feeds B1 early)
  B1:  AllReduce-max qp within 4-core batch group (issued immediately)
  P1b: ALL v' GEMMs (x stationary -> v'T in SBUF fp16); the pe bias enters
       via one K=32 matmul per pair (zselw x w2pe2) accumulated in PSUM.
       Runs concurrently with the B1 collective -> PE never stalls on it.
  M:   offsets/attn/softmax -> 32x32 mixing matrix M -> block-diag w128
  P1c: ALL mix matmuls (z-on-partitions block-diag M.T); ACT copies
       PSUM->vbuf fp16; DVE bn_stats on the fp16 copy
  B2:  AllReduce-add BN stats over 8 cores; fold scale/shift
  P2b: normalize (ACT/DVE split) + residual add + fp16 DMA out
"""
from contextlib import ExitStack

import numpy as np

import concourse.bass as bass
import concourse.bacc as bacc
import concourse.tile as tile
import concourse.mybir as mybir
from concourse import bass_utils

F32 = mybir.dt.float32
F16 = mybir.dt.float16

B, C, Z, H, W = 2, 256, 32, 48, 48
HL = H // 4            # h rows per core
FL = HL * W            # 576 hw positions per core
FPT = 16               # hw positions per tile
G = FL // FPT          # 36 tiles
NHP = 6                # heads*points
NLOC = FL * Z          # per-core elements per channel (18432)
NTOT = 8 * NLOC        # global elements per channel (147456)
EPS = 1e-5

QP_GROUPS = [[0, 1, 2, 3], [4, 5, 6, 7]]
ALL_GROUP = [[0, 1, 2, 3, 4, 5, 6, 7]]


def ts(i, n):
    return slice(i * n, (i + 1) * n)


def _emit(tc, t, dbg=False):
    nc = tc.nc
    ctx = ExitStack()
    cp = ctx.enter_context(tc.tile_pool(name="consts", bufs=1))
    bigp = ctx.enter_context(tc.tile_pool(name="big", bufs=1))
    smp = ctx.enter_context(tc.tile_pool(name="smalls", bufs=3))
    resp = ctx.enter_context(tc.tile_pool(name="res", bufs=3))
    dramp = ctx.enter_context(tc.tile_pool(name="dram", bufs=1, space="DRAM"))

    # ---- constant loads ----
    qwT = cp.tile([128, 2, C], F16)
    nc.sync.dma_start(qwT[:], t["qwT16"].rearrange("(h p) m -> p h m", p=128))
    w2T = cp.tile([128, 2, C], F16)
    nc.sync.dma_start(w2T[:], t["w2T16"].rearrange("(h p) m -> p h m", p=128))
    qpe = cp.tile([128, 2, Z], F32)
    nc.sync.dma_start(qpe[:], t["qpe"].rearrange("(h p) z -> p h z", p=128))
    sawT = cp.tile([128, 2, 12], F16)
    nc.sync.dma_start(sawT[:], t["sawT16"].rearrange("(h p) j -> p h j", p=128))
    ident = cp.tile([Z, Z], F16)
    nc.sync.dma_start(ident[:], t["ident32"][:])
    # zero-padded to K=128 so the PE tile config matches the x-chains
    zselw = cp.tile([128, 128], F16)
    nc.sync.dma_start(zselw[:], t["zselw"][:])
    w2pe2 = cp.tile([128, 2 * C], F16)
    nc.sync.dma_start(w2pe2[:], t["w2pe2"][:])
    iota = cp.tile([Z, Z], F32)
    nc.sync.dma_start(iota[:], t["iota"][:])
    sab = cp.tile([Z, 12], F32)
    sab_src = bass.AP(tensor=t["sab"].tensor, offset=0, ap=[[0, Z], [1, 12]])
    nc.sync.dma_start(sab[:], sab_src)
    gam = cp.tile([128, 2, 1], F32)
    nc.sync.dma_start(gam[:], t["gb"].rearrange("(h p) j -> p h j", p=128)[:, :, 0:1])
    bet = cp.tile([128, 2, 1], F32)
    nc.sync.dma_start(bet[:], t["gb"].rearrange("(h p) j -> p h j", p=128)[:, :, 1:2])

    # big persistent regions
    x_all = bigp.tile([128, 2, G, FPT * Z], F16)
    vbuf = bigp.tile([128, 2, G, FPT * Z], F16)
    for gp in range(G // 2):
        for kh in range(2):
            nc.sync.dma_start(
                x_all[:, kh, ts(gp, 2), :],
                t["x16"][ts(kh, 128), ts(gp, 2 * FPT), :],
            )

    qmax = cp.tile([128, 2, Z], F32)
    nc.vector.memset(qmax[:], -3.0e38)
    w128 = cp.tile([128, 128], F16)
    nc.vector.memset(w128[:], 0.0)
    stats = cp.tile([128, 2, G, 6], F32)

    # ---- P1a: q GEMMs -> per-z max (feeds the qp collective early) ----
    with tc.tile_pool(name="p1q", bufs=2, space="PSUM") as p1q:
        for g in range(G):
            qps = p1q.tile([128, 2, FPT * Z], F32, tag="qps")
            # kh outer: consecutive matmuls hit alternating PSUM banks so the
            # PE can pipeline the next stationary load under the current MM
            for kh in range(2):
                for mh in range(2):
                    nc.tensor.matmul(
                        qps[:, mh, :], qwT[:, kh, ts(mh, 128)], x_all[:, kh, g, :],
                        start=(kh == 0), stop=(kh == 1),
                        skip_group_check=True,
                    )
            qred = smp.tile([128, 2, Z], F32, tag="qred")
            nc.vector.tensor_reduce(
                out=qred[:],
                in_=qps[:].rearrange("p h (f z) -> p h z f", z=Z),
                axis=mybir.AxisListType.X,
                op=mybir.AluOpType.max,
            )
            nc.vector.tensor_max(qmax[:], qmax[:], qred[:])

    # ---- B1 issue: qp allreduce (hidden under the v' GEMMs below) ----
    qp_loc = cp.tile([128, 2, Z], F32)
    nc.vector.tensor_add(qp_loc[:], qmax[:], qpe[:])
    qpin = dramp.tile([128, 2, Z], F32)
    qpout = dramp.tile([128, 2, Z], F32)
    nc.sync.dma_start(qpin[:], qp_loc[:])
    nc.gpsimd.collective_compute(
        "AllReduce", mybir.AluOpType.max, replica_groups=QP_GROUPS,
        ins=[qpin[:].opt()], outs=[qpout[:].opt()],
    )
    qp16 = cp.tile([128, 2, Z], F16)
    nc.gpsimd.dma_start(qp16[:], qpout[:])

    # ---- P1b: ALL v' GEMMs (independent of B1) ----
    pv = ctx.enter_context(tc.tile_pool(name="pv", bufs=2, space="PSUM"))
    for g in range(G):
        # both pair-chains live at once; instructions interleave across the
        # two PSUM banks so stationary loads pipeline under the previous MM
        vps = [
            pv.tile([128, 2, C], F32, tag=f"vps{pair}", name=f"vps{pair}")
            for pair in range(2)
        ]
        for pair in range(2):
            # W2@pe bias first (z-pattern repeats every 32 partitions):
            # start=True zeroes the full region, x-chains then accumulate.
            nc.tensor.matmul(
                vps[pair][:], zselw[:], w2pe2[:],
                start=True, stop=False, skip_group_check=True,
            )
        for i in range(2):
            for kh in range(2):
                for pair in range(2):
                    blk = pair * 2 + i
                    nc.tensor.matmul(
                        vps[pair][:, i, :], x_all[:, kh, g, ts(blk, 128)],
                        w2T[:, kh, :], start=False,
                        stop=(i == 1 and kh == 1),
                        skip_group_check=True,
                    )
        for pair in range(2):
            out_view = vbuf[:, :, g, ts(pair, 2 * 128)].rearrange(
                "p h (b c) -> p b h c", b=2
            )
            in_view = vps[pair][:].rearrange("p b (h c) -> p b h c", h=2)
            if pair == 0:
                nc.vector.tensor_copy(out_view, in_view)
            else:
                nc.scalar.copy(out_view, in_view)
        if dbg and g == 0:
            nc.sync.dma_start(t["dbg_v0"][:], vbuf[:, :, 0, :])

    # ---- M build (after B1): offsets/attn -> M -> block-diag w128 ----
    psml_ctx = tc.tile_pool(name="psml", bufs=1, space="PSUM")
    psml = psml_ctx.__enter__()
    sa_ps = psml.tile([Z, 12], F32, tag="small")
    for kh in range(2):
        nc.tensor.matmul(
            sa_ps[:], qp16[:, kh, :], sawT[:, kh, :],
            start=(kh == 0), stop=(kh == 1),
        )
    logits = cp.tile([Z, 12], F32)
    nc.vector.tensor_add(logits[:], sa_ps[:], sab[:])
    off = cp.tile([Z, NHP], F32)
    nc.vector.tensor_scalar(
        out=off[:], in0=logits[:, 0:NHP], scalar1=0.0, scalar2=float(Z - 1),
        op0=mybir.AluOpType.max, op1=mybir.AluOpType.min,
    )
    ex = cp.tile([Z, NHP], F32)
    nc.scalar.activation(ex[:], logits[:, NHP:12], mybir.ActivationFunctionType.Exp)
    ssum = cp.tile([Z, 1], F32)
    nc.vector.tensor_reduce(
        out=ssum[:], in_=ex[:], axis=mybir.AxisListType.X, op=mybir.AluOpType.add
    )
    rinv = cp.tile([Z, 1], F32)
    nc.vector.reciprocal(rinv[:], ssum[:])
    attn = cp.tile([Z, NHP], F32)
    nc.vector.tensor_scalar_mul(attn[:], ex[:], rinv[:, 0:1])

    # M[z, y] = sum_p attn[z,p] * relu(1 - |off[z,p] - y|)  (linear interp hat)
    msb = cp.tile([Z, Z], F32)
    mtmp = cp.tile([Z, Z], F32)
    dmy = cp.tile([Z, Z], F32)
    um = cp.tile([Z, Z], F32)
    vm = cp.tile([Z, Z], F32)
    for p in range(NHP):
        nc.vector.tensor_scalar(
            out=dmy[:], in0=iota[:], scalar1=off[:, p:p + 1], scalar2=None,
            op0=mybir.AluOpType.subtract,
        )
        nc.vector.tensor_scalar(
            out=um[:], in0=dmy[:], scalar1=-1.0, scalar2=1.0,
            op0=mybir.AluOpType.mult, op1=mybir.AluOpType.add,
        )
        nc.vector.tensor_scalar_add(vm[:], dmy[:], 1.0)
        nc.vector.tensor_tensor(
            out=um[:], in0=um[:], in1=vm[:], op=mybir.AluOpType.min
        )
        dst = msb if p == 0 else mtmp
        nc.vector.tensor_scalar(
            out=dst[:], in0=um[:], scalar1=0.0, scalar2=attn[:, p:p + 1],
            op0=mybir.AluOpType.max, op1=mybir.AluOpType.mult,
        )
        if p > 0:
            nc.vector.tensor_add(msb[:], msb[:], mtmp[:])
    m16 = cp.tile([Z, Z], F16)
    nc.vector.tensor_copy(m16[:], msb[:])
    mt_ps = psml.tile([Z, Z], F16, tag="small")
    nc.tensor.transpose(mt_ps[:], m16[:], ident[:])
    mt16 = cp.tile([Z, Z], F16)
    nc.vector.tensor_copy(mt16[:], mt_ps[:])
    for b4 in range(4):
        nc.vector.tensor_copy(w128[ts(b4, 32), ts(b4, 32)], mt16[:])
    psml_ctx.__exit__(None, None, None)

    # ---- P1c: ALL mix matmuls; ACT copy; DVE stats on fp16 copy ----
    pmix = ctx.enter_context(tc.tile_pool(name="pmix", bufs=2, space="PSUM"))
    for g in range(G):
        mix = pmix.tile([128, 2, FPT * Z], F32, tag="mix")
        for h in range(2):
            for blk in range(4):
                nc.tensor.matmul(
                    mix[:, h, ts(blk, 128)], vbuf[:, h, g, ts(blk, 128)],
                    w128[:], start=True, stop=True,
                )
        nc.scalar.copy(vbuf[:, :, g, :], mix[:])
        for h in range(2):
            nc.vector.bn_stats(stats[:, h, g, :], vbuf[:, h, g, :])
        if dbg and g == 0:
            nc.sync.dma_start(t["dbg_mix0"][:], vbuf[:, :, 0, :])

    # ---- B2: global BN stats ----
    mv = cp.tile([128, 2, 2], F32)
    for h in range(2):
        nc.vector.bn_aggr(mv[:, h, :], stats[:, h, :, :])
    msq = cp.tile([128, 2, 1], F32)
    nc.vector.tensor_mul(msq[:], mv[:, :, 0:1], mv[:, :, 0:1])
    ex2 = cp.tile([128, 2, 1], F32)
    nc.vector.tensor_add(ex2[:], mv[:, :, 1:2], msq[:])
    red_in = cp.tile([128, 2, 2], F32)
    nc.vector.tensor_scalar_mul(red_in[:, :, 0:1], mv[:, :, 0:1], float(NLOC))
    nc.vector.tensor_scalar_mul(red_in[:, :, 1:2], ex2[:], float(NLOC))
    rin = dramp.tile([128, 2, 2], F32)
    rout = dramp.tile([128, 2, 2], F32)
    nc.sync.dma_start(rin[:], red_in[:])
    nc.gpsimd.collective_compute(
        "AllReduce", mybir.AluOpType.add, replica_groups=ALL_GROUP,
        ins=[rin[:].opt()], outs=[rout[:].opt()],
    )
    gst = cp.tile([128, 2, 2], F32)
    nc.sync.dma_start(gst[:], rout[:])
    mean_g = cp.tile([128, 2, 1], F32)
    nc.vector.tensor_scalar_mul(mean_g[:], gst[:, :, 0:1], 1.0 / NTOT)
    ex2g = cp.tile([128, 2, 1], F32)
    nc.vector.tensor_scalar_mul(ex2g[:], gst[:, :, 1:2], 1.0 / NTOT)
    m2g = cp.tile([128, 2, 1], F32)
    nc.vector.tensor_mul(m2g[:], mean_g[:], mean_g[:])
    var_g = cp.tile([128, 2, 1], F32)
    nc.vector.tensor_sub(var_g[:], ex2g[:], m2g[:])
    eps_sb = cp.tile([128, 1], F32)
    nc.vector.memset(eps_sb[:], EPS)
    sd = cp.tile([128, 2, 1], F32)
    nc.scalar.activation(
        sd[:], var_g[:], mybir.ActivationFunctionType.Sqrt, bias=eps_sb[:]
    )
    rs = cp.tile([128, 2, 1], F32)
    nc.vector.reciprocal(rs[:], sd[:])
    s_sb = cp.tile([128, 2, 1], F32)
    nc.vector.tensor_mul(s_sb[:], rs[:], gam[:])
    ms_t = cp.tile([128, 2, 1], F32)
    nc.vector.tensor_mul(ms_t[:], mean_g[:], s_sb[:])
    t_sb = cp.tile([128, 2, 1], F32)
    nc.vector.tensor_sub(t_sb[:], bet[:], ms_t[:])
    if dbg:
        nc.sync.dma_start(t["dbg_qp"][:], qp16[:])
        nc.sync.dma_start(t["dbg_m"][:], msb[:])
        nc.sync.dma_start(t["dbg_w128"][:], w128[:])
        nc.sync.dma_start(t["dbg_s"][:], s_sb[:])
        nc.sync.dma_start(t["dbg_t"][:], t_sb[:])
        nc.sync.dma_start(t["dbg_mv"][:], mv[:])

    # ---- P2b: normalize (ACT/DVE) + residual (DVE) + fp16 store ----
    out_v = t["out"].rearrange("(h p) f z -> p h f z", p=128)
    for g in range(G):
        nrm = resp.tile([128, 2, FPT * Z], F16, tag="nrm")
        for h in range(2):
            if (g + h) % 2 == 0:
                nc.scalar.activation(
                    nrm[:, h, :], vbuf[:, h, g, :],
                    mybir.ActivationFunctionType.Identity,
                    scale=s_sb[:, h, :], bias=t_sb[:, h, :],
                )
            else:
                nc.vector.tensor_scalar(
                    out=nrm[:, h, :], in0=vbuf[:, h, g, :],
                    scalar1=s_sb[:, h, :], scalar2=t_sb[:, h, :],
                    op0=mybir.AluOpType.mult, op1=mybir.AluOpType.add,
                )
        res = resp.tile([128, 2, FPT * Z], F16, tag="res")
        nc.vector.tensor_add(res[:], nrm[:], x_all[:, :, g, :])
        for h in range(2):
            nc.sync.dma_start(
                out_v[:, h, ts(g, FPT), :],
                res[:, h, :].rearrange("p (f z) -> p f z", z=Z),
            )
    ctx.close()


_BUILT = None


def _build(dbg=False):
    global _BUILT
    if _BUILT is not None and not dbg:
        return _BUILT
    nc = bacc.Bacc("TRN2", target_bir_lowering=False, debug=False, num_devices=8)
    t = {}
    t["x16"] = nc.dram_tensor("x16", [C, FL, Z], F16, kind="ExternalInput").ap()
    t["qwT16"] = nc.dram_tensor("qwT16", [C, C], F16, kind="ExternalInput").ap()
    t["w2T16"] = nc.dram_tensor("w2T16", [C, C], F16, kind="ExternalInput").ap()
    t["qpe"] = nc.dram_tensor("qpe", [C, Z], F32, kind="ExternalInput").ap()
    t["sawT16"] = nc.dram_tensor("sawT16", [C, 12], F16, kind="ExternalInput").ap()
    t["sab"] = nc.dram_tensor("sab", [12], F32, kind="ExternalInput").ap()
    t["iota"] = nc.dram_tensor("iota", [Z, Z], F32, kind="ExternalInput").ap()
    t["ident32"] = nc.dram_tensor("ident32", [Z, Z], F16, kind="ExternalInput").ap()
    t["zselw"] = nc.dram_tensor("zselw", [128, 128], F16, kind="ExternalInput").ap()
    t["w2pe2"] = nc.dram_tensor("w2pe2", [128, 2 * C], F16, kind="ExternalInput").ap()
    t["gb"] = nc.dram_tensor("gb", [C, 2], F32, kind="ExternalInput").ap()
    t["out"] = nc.dram_tensor("out", [C, FL, Z], F16, kind="ExternalOutput").ap()
    if dbg:
        t["dbg_v0"] = nc.dram_tensor("dbg_v0", [128, 2, FPT * Z], F16, kind="ExternalOutput").ap()
        t["dbg_mix0"] = nc.dram_tensor("dbg_mix0", [128, 2, FPT * Z], F16, kind="ExternalOutput").ap()
        t["dbg_qp"] = nc.dram_tensor("dbg_qp", [128, 2, Z], F16, kind="ExternalOutput").ap()
        t["dbg_m"] = nc.dram_tensor("dbg_m", [Z, Z], F32, kind="ExternalOutput").ap()
        t["dbg_w128"] = nc.dram_tensor("dbg_w128", [128, 128], F16, kind="ExternalOutput").ap()
        t["dbg_s"] = nc.dram_tensor("dbg_s", [128, 2, 1], F32, kind="ExternalOutput").ap()
        t["dbg_t"] = nc.dram_tensor("dbg_t", [128, 2, 1], F32, kind="ExternalOutput").ap()
        t["dbg_mv"] = nc.dram_tensor("dbg_mv", [128, 2, 2], F32, kind="ExternalOutput").ap()
    with tile.TileContext(nc) as tc:
        _emit(tc, t, dbg=dbg)
    nc.compile()
    if not dbg:
        _BUILT = nc
    return nc


def _make_pe():
    pos = np.arange(Z, dtype=np.float32)[:, None]
    div = np.exp(np.arange(0, C, 2, dtype=np.float32) * (-np.log(10000.0) / C))
    pe = np.zeros((Z, C), dtype=np.float32)
    pe[:, 0::2] = np.sin(pos * div)
    pe[:, 1::2] = np.cos(pos * div)
    return pe


def _prepare_in_maps(features, q_w, v_w, o_w, offs_w, offs_b, attn_w, attn_b,
                     gamma, beta):
    features = np.ascontiguousarray(np.asarray(features, dtype=np.float32))
    pe = _make_pe()  # (Z, C) f32
    q_w = np.asarray(q_w, dtype=np.float32)
    W2 = np.asarray(o_w, dtype=np.float32) @ np.asarray(v_w, dtype=np.float32)
    w2pe = pe @ W2.T  # (Z, C)
    shared = {
        "qwT16": np.ascontiguousarray(q_w.T).astype(np.float16),
        "w2T16": np.ascontiguousarray(W2.T).astype(np.float16),
        "qpe": np.ascontiguousarray(q_w @ pe.T),
        "sawT16": np.concatenate(
            [np.asarray(offs_w).T, np.asarray(attn_w).T], axis=1
        ).astype(np.float16),
        "sab": np.concatenate(
            [np.asarray(offs_b), np.asarray(attn_b)]
        ).astype(np.float32),
        "iota": np.tile(np.arange(Z, dtype=np.float32)[None, :], (Z, 1)),
        "ident32": np.eye(Z, dtype=np.float16),
        "zselw": np.vstack(
            [np.tile(np.eye(Z, dtype=np.float16), (1, 128 // Z)),
             np.zeros((128 - Z, 128), np.float16)]
        ),
        "w2pe2": np.vstack(
            [np.tile(w2pe, (1, 2)).astype(np.float16),
             np.zeros((128 - Z, 2 * C), np.float16)]
        ),
        "gb": np.stack(
            [np.asarray(gamma, np.float32), np.asarray(beta, np.float32)], axis=1
        ),
    }
    in_maps = []
    for k in range(8):
        bi, hq = k // 4, k % 4
        xs = features[bi][:, :, hq * HL:(hq + 1) * HL, :]
        xs = np.ascontiguousarray(xs.transpose(0, 2, 3, 1)).reshape(C, FL, Z)
        m = dict(shared)
        m["x16"] = xs.astype(np.float16)
        in_maps.append(m)
    return in_maps


def kernel(**inputs):
    nc = _build()
    in_maps = _prepare_in_maps(**inputs)
    res = bass_utils.run_bass_kernel_spmd(nc, in_maps, core_ids=list(range(8)))

    full = np.empty((B, C, Z, H, W), dtype=np.float32)
    for k in range(8):
        bi, hq = k // 4, k % 4
        o = res.results[k]["out"].astype(np.float32).reshape(C, HL, W, Z).transpose(0, 3, 1, 2)
        full[bi][:, :, hq * HL:(hq + 1) * HL, :] = o
    return full
